# revision 12
# baseline (speedup 1.0000x reference)
"""Capsule-routing kernel v3 — batch-merged, parity-packed, wave-pipelined.

Per core: 8 batches in 2 groups of 4, processed in lockstep so that matmuls
merge across batches and vector/scalar ops run at full [128, *] width.

Index conventions (per group of GB=4 batches):
  capsule n = 2*kc + parity   (kc in [0,16), parity in {0,1})
  slot(j, kc) = j*16 + kc     in [0, 64)
  p'(j, n)  = parity*64 + slot  -> o-all / oT / z column order  (parity-major)
  b/c layout: [128 (parity*64+G), 32 g, 64 slot]
Row-packed MM pairs: even capsules use partitions 0:64, odd 64:128, running
concurrently in distinct PE row groups (bf16 only).

v3 changes vs v2:
  - zstep: dup-column weights (wt2c [64 d, 32 g, 128 (dup,i)]) -> one N=128
    matmul per g writes both partition halves of z2; FWL-eligible 128-col LDW.
  - squash: f = exp(0.5*ln(s') - ln(1+s')) so the Act engine stays on the
    ln/exp/square table (no ACT_TABLE_LOAD thrash); f-scale fused into the
    PSUM->SBUF copy via activation(Copy, scale=f).
  - softmax/pstep wave-pipelined per batch j: exp/reduce/fold/recip/c-mul/
    pstep per 16-slot wave so the 2us exp is off the critical path.
  - bf16 expb/T/e2 (2x DVE reduce, 1-cycle fold MMs).
  - engine rebalance: PSUM->SBUF copies + b-adds spread over Vec/Scalar/Pool.
  - DMA: group-0 inputs first on sync queue, consts on gpsimd queue,
    one output DMA per group.
"""

import numpy as np

B, IN_CAPS, IN_DIM = 64, 2048, 64
NUM, DIM = 32, 64
N_CORES = 8
BPC = B // N_CORES  # 8 batches per core
GB = 4              # batches per merged group
NG = BPC // GB      # 2 groups
EPS = 1e-7

_CACHE = {}


def _build_nc(bpc=BPC):
    import concourse.bacc as bacc
    import concourse.tile as tile
    from concourse import mybir

    f32 = mybir.dt.float32
    bf16 = mybir.dt.bfloat16
    Act = mybir.ActivationFunctionType
    Alu = mybir.AluOpType

    ng = bpc // GB
    nc = bacc.Bacc("TRN2", target_bir_lowering=False, debug=False, num_devices=N_CORES)

    # ---- DRAM I/O (per-core shapes) ----
    # x[b, parity*64+G, kc, i] = X[b, (2kc+parity)*64+G, i]
    x_d = nc.dram_tensor("x", [bpc, 128, 16, IN_DIM], bf16, kind="ExternalInput")
    # xt2[b, q, r] = X[b, r, q % 64]   (stacked twice for row-packing)
    xt2_d = nc.dram_tensor("xt2", [bpc, 128, IN_CAPS], bf16, kind="ExternalInput")
    # xs[grp, i, p'] = sum_G X[b, n*64+G, i] at p' = parity*64 + j*16 + kc
    xs_d = nc.dram_tensor("xs", [ng, IN_DIM, 128], bf16, kind="ExternalInput")
    # w2 = W stacked twice: [128 (dup i), 2048]
    w2_d = nc.dram_tensor("w2", [128, NUM * DIM], bf16, kind="ExternalInput")
    # wt2c[d, g, dup*64+i] = W[i, g*64+d]  (columns duplicated)
    wt2c_d = nc.dram_tensor("wt2c", [IN_DIM, 32, 128], bf16, kind="ExternalInput")
    wsum_d = nc.dram_tensor("wsum", [IN_DIM, DIM], bf16, kind="ExternalInput")
    i128_d = nc.dram_tensor("i128", [128, 128], bf16, kind="ExternalInput")
    # ef[q, p] = 1 if q == p (mod 64): one matmul folds the parity halves of
    # T AND broadcasts the full softmax denominator to all 128 partitions
    ef_d = nc.dram_tensor("ef", [128, 128], bf16, kind="ExternalInput")
    out_d = nc.dram_tensor("out", [bpc, NUM, DIM], f32, kind="ExternalOutput")

    with tile.TileContext(nc) as tc:
        with (
            tc.tile_pool(name="const", bufs=1) as cpool,
            tc.tile_pool(name="inp", bufs=2) as ipool,
            tc.tile_pool(name="work", bufs=2) as wpool,
            tc.tile_pool(name="big", bufs=3) as bigpool,
            tc.tile_pool(name="ps_wave", bufs=2, space="PSUM") as ps_wave,
            tc.tile_pool(name="ps_z", bufs=2, space="PSUM") as ps_z,
            tc.tile_pool(name="ps_db", bufs=2, space="PSUM") as ps_db,
            tc.tile_pool(name="ps_o", bufs=1, space="PSUM") as ps_o,
            tc.tile_pool(name="ps_sm", bufs=1, space="PSUM") as ps_sm,
        ):
            # ---- input DMAs: group 0 first (sync queue), consts on gpsimd ----
            xs_t = [None] * ng
            xt2_t = [[None] * GB for _ in range(ng)]
            x_t = [[None] * GB for _ in range(ng)]
            wsum_t = cpool.tile([IN_DIM, DIM], bf16, tag="wsum")
            i128_t = cpool.tile([128, 128], bf16, tag="i128")
            wt2c_t = cpool.tile([IN_DIM, 32, 128], bf16, tag="wt2c")
            ef_t = cpool.tile([128, 128], bf16, tag="ef")
            w2_t = cpool.tile([128, NUM * DIM], bf16, tag="w2")

            def load_group(grp):
                for j in range(GB):
                    b = grp * GB + j
                    t = ipool.tile([128, IN_CAPS], bf16, tag=f"xt2_{grp}_{j}", name=f"xt2_{grp}_{j}")
                    nc.sync.dma_start(t[:], xt2_d[b])
                    xt2_t[grp][j] = t
                for j in range(GB):
                    b = grp * GB + j
                    t = ipool.tile([128, 16, IN_DIM], bf16, tag=f"x_{grp}_{j}", name=f"x_{grp}_{j}")
                    nc.sync.dma_start(t[:], x_d[b])
                    x_t[grp][j] = t

            nc.sync.dma_start(wsum_t[:], wsum_d[:])
            for grp in range(ng):
                xs_t[grp] = ipool.tile([IN_DIM, 128], bf16, tag=f"xs{grp}", name=f"xs{grp}")
                nc.sync.dma_start(xs_t[grp][:], xs_d[grp])
            load_group(0)
            # consts on the gpsimd queue so the sync queue stays on group data
            nc.gpsimd.dma_start(i128_t[:], i128_d[:])
            nc.gpsimd.dma_start(wt2c_t[:], wt2c_d[:])
            nc.gpsimd.dma_start(ef_t[:], ef_d[:])
            nc.gpsimd.dma_start(w2_t[:], w2_d[:])
            load_group(1)

            eps_t = cpool.tile([128, 1], f32, tag="eps")
            nc.vector.memset(eps_t[:], EPS)
            one_eps_t = cpool.tile([128, 1], f32, tag="one_eps")
            nc.vector.memset(one_eps_t[:], 1.0 + EPS)

            # ---------------- squash ----------------
            def squash(o_ps, want_f32=False):
                """psum [128,64] -> (f32 sbuf or None, bf16 sbuf) squashed.

                f = sqrt(s')/(1+s') = exp(0.5*ln(s'+eps') - ln(1+s')); all Act
                ops stay in the ln/exp/square table set.
                """
                scr = wpool.tile([128, DIM], bf16, tag="scr")
                s0 = wpool.tile([128, 1], f32, tag="s0")
                nc.scalar.activation(scr[:], o_ps[:], Act.Square, accum_out=s0[:])
                la = wpool.tile([128, 1], f32, tag="la")
                nc.scalar.activation(la[:], s0[:], Act.Ln, bias=eps_t[:])
                lb = wpool.tile([128, 1], f32, tag="lb")
                nc.scalar.activation(lb[:], s0[:], Act.Ln, bias=one_eps_t[:])
                t = wpool.tile([128, 1], f32, tag="t")
                nc.vector.tensor_scalar(t[:], la[:], 0.5, lb[:], Alu.mult, Alu.subtract)
                f = wpool.tile([128, 1], f32, tag="f")
                nc.scalar.activation(f[:], t[:], Act.Exp)
                o_f32 = None
                if want_f32:
                    o_f32 = wpool.tile([128, DIM], f32, tag="osqf")
                    nc.scalar.activation(o_f32[:], o_ps[:], Act.Copy, scale=f[:])
                o_bf = wpool.tile([128, DIM], bf16, tag="osqb")
                nc.scalar.activation(o_bf[:], o_ps[:], Act.Copy, scale=f[:])
                return o_f32, o_bf

            def transpose_o(o_bf):
                """[128,64] bf16 -> oT sbuf [64,128] bf16."""
                t_ps = ps_wave.tile([IN_DIM, 128], bf16, tag="pw")
                nc.tensor.transpose(t_ps[:], o_bf[:], i128_t[:])
                oT = wpool.tile([IN_DIM, 128], bf16, tag="oT")
                nc.vector.tensor_copy(oT[:], t_ps[:])
                return oT

            def zstep(oT):
                """oT [64,128] -> z2 sbuf [128 (dup i), 32 g, 128 p'] bf16."""
                z2 = bigpool.tile([128, 32, 128], bf16, tag="z2")
                for gw in range(8):  # waves of 4 g
                    z_ps = ps_z.tile([128, 4, 128], f32, tag="z")
                    for j in range(4):
                        g = gw * 4 + j
                        nc.tensor.matmul(
                            z_ps[:, j, :], lhsT=wt2c_t[:, g, :], rhs=oT[:],
                            start=True, stop=True,
                        )
                    dst = z2[:, gw * 4 : gw * 4 + 4, :]
                    if gw % 2 == 0:
                        nc.scalar.copy(dst, z_ps[:])
                    else:
                        nc.vector.tensor_copy(dst, z_ps[:])
                return z2

            def dbstep(z2, xt2_g, b_prev):
                """-> new b sbuf [128, 32 g, 64 slot] f32 (written per wave)."""
                nb = bigpool.tile([128, 32, 64], f32, tag="b")
                for j in range(GB):  # one wave per batch: 16 slots
                    db_ps = ps_db.tile([128, 32, 16], f32, tag="db")
                    for kc in range(16):
                        slot = j * 16 + kc
                        for parity in range(2):
                            h = parity * 64
                            n = 2 * kc + parity
                            nc.tensor.matmul(
                                db_ps[h : h + 64, :, kc],
                                lhsT=xt2_g[j][h : h + 64, n * 64 : (n + 1) * 64],
                                rhs=z2[h : h + 64, :, h + slot],
                                start=True, stop=True,
                            )
                    dst = nb[:, :, j * 16 : (j + 1) * 16]
                    if b_prev is None:
                        if j % 2 == 0:
                            nc.vector.tensor_copy(dst, db_ps[:])
                        else:
                            nc.scalar.copy(dst, db_ps[:])
                    elif j % 2 == 0:
                        nc.vector.tensor_add(
                            dst, b_prev[:, :, j * 16 : (j + 1) * 16], db_ps[:]
                        )
                    else:
                        # gpsimd cannot read PSUM: stage via a scalar copy
                        stg = wpool.tile([128, 32, 16], f32, tag=f"dbs{j}")
                        nc.scalar.copy(stg[:], db_ps[:])
                        nc.gpsimd.tensor_add(
                            dst, b_prev[:, :, j * 16 : (j + 1) * 16], stg[:]
                        )
                return nb

            def smp_step(b_sb, x_g):
                """softmax + pstep, wave-pipelined per batch j.

                b [128, 32 g, 64 slot] f32 -> p_all [128, 64 slot, 32 g] bf16.
                """
                expb = bigpool.tile([128, 32, 64], bf16, tag="expb")
                T = wpool.tile([128, GB, 32], bf16, tag="T")
                rs2 = wpool.tile([128, GB, 32], f32, tag="rs2")
                p_all = bigpool.tile([128, 64, 32], bf16, tag="pall")
                mul_eng = [nc.vector, nc.gpsimd, nc.vector, nc.gpsimd]
                for j in range(GB):
                    sl = slice(j * 16, (j + 1) * 16)
                    # exp of this batch's 16 slots (all g): [128, 32, 16]
                    nc.scalar.activation(expb[:, :, sl], b_sb[:, :, sl], Act.Exp)
                    # T[p, j, g] = sum_kc expb[p, g, j*16+kc]  (bf16 out: the
                    # 0.4% rounding on the softmax denominator is within budget
                    # and enables the 2x 16-bit DVE mode)
                    with nc.allow_low_precision(reason="softmax denom bf16"):
                        nc.vector.tensor_reduce(
                            T[:, j, :].unsqueeze(-1).squeeze(-1),
                            expb[:, :, sl],
                            mybir.AxisListType.X, Alu.add,
                        )
                    # S2[p, g] = sum over both parity halves, broadcast to
                    # all 128 partitions in one matmul
                    S2_ps = ps_sm.tile([128, 32], f32, tag="sm")
                    nc.tensor.matmul(
                        S2_ps[:], lhsT=ef_t[:], rhs=T[:, j, :],
                        start=True, stop=True,
                    )
                    nc.vector.reciprocal(rs2[:, j, :], S2_ps[:])
                    # c = expb * 1/S2 (broadcast over kc), in place
                    mul_eng[j].tensor_mul(
                        expb[:, :, sl],
                        expb[:, :, sl],
                        rs2[:, j, :, None].to_broadcast([128, 32, 16]),
                    )
                    # pstep for this batch's 16 slots
                    p_ps = ps_wave.tile([128, 16, 32], f32, tag="pw")
                    for kc in range(16):
                        slot = j * 16 + kc
                        for parity in range(2):
                            h = parity * 64
                            nc.tensor.matmul(
                                p_ps[h : h + 64, kc, :],
                                lhsT=x_g[j][h : h + 64, kc, :],
                                rhs=expb[h : h + 64, :, slot],
                                start=True, stop=True,
                            )
                    if j % 2 == 0:
                        nc.scalar.copy(p_all[:, sl, :], p_ps[:])
                    else:
                        nc.vector.tensor_copy(p_all[:, sl, :], p_ps[:])
                return p_all

            def mm2(p_all, o_ps):
                # MM2: per g, row-packed even/odd halves accumulate into o-all.
                for g in range(32):
                    nc.tensor.matmul(
                        o_ps[0:64, :],
                        lhsT=p_all[0:64, :, g],
                        rhs=w2_t[0:64, g * 64 : (g + 1) * 64],
                        start=(g == 0), stop=(g == 31),
                        skip_group_check=True,
                    )
                    nc.tensor.matmul(
                        o_ps[64:128, :],
                        lhsT=p_all[64:128, :, g],
                        rhs=w2_t[64:128, g * 64 : (g + 1) * 64],
                        start=(g == 0), stop=(g == 31),
                        skip_group_check=True,
                    )

            # ================= interleaved group emission =================
            st = [dict() for _ in range(ng)]

            def ph_iter0(g_):
                s_ = st[g_]
                o_ps = ps_o.tile([128, DIM], f32, tag="o")
                nc.tensor.matmul(o_ps[:], lhsT=xs_t[g_][:], rhs=wsum_t[:], start=True, stop=True)
                s_["o_f32"], s_["o_bf"] = squash(o_ps)
                s_["b"] = None
                s_["m2s_count"] = 0

            def ph_tz(g_):
                s_ = st[g_]
                oT = transpose_o(s_["o_bf"])
                s_["z2"] = zstep(oT)

            def ph_db(g_):
                s_ = st[g_]
                s_["b"] = dbstep(s_["z2"], xt2_t[g_], s_["b"])

            def ph_smp(g_):
                s_ = st[g_]
                s_["pall"] = smp_step(s_["b"], x_t[g_])

            def ph_m2s(g_):
                s_ = st[g_]
                o_ps = ps_o.tile([128, DIM], f32, tag="o")
                mm2(s_["pall"], o_ps)
                s_["m2s_count"] += 1
                s_["o_f32"], s_["o_bf"] = squash(o_ps, want_f32=(s_["m2s_count"] == 2))

            def ph_out(g_):
                grp, s_ = g_, st[g_]
                # one DMA per group: sbuf [128 (par, j, kc), 64] -> dram view
                dst = out_d[grp * GB : (grp + 1) * GB].rearrange(
                    "b (kc par) d -> par b kc d", par=2
                )
                nc.gpsimd.dma_start(dst, s_["o_f32"][:])

            phases = [ph_iter0, ph_tz, ph_db, ph_smp, ph_m2s,
                      ph_tz, ph_db, ph_smp, ph_m2s, ph_out]
            OFFSET = 1
            for k in range(len(phases) + OFFSET * (ng - 1)):
                for grp in range(ng):
                    kk = k - OFFSET * grp
                    if 0 <= kk < len(phases):
                        phases[kk](grp)

    nc.compile()
    return nc


def _get_nc():
    if "nc" not in _CACHE:
        _CACHE["nc"] = _build_nc()
    return _CACHE["nc"]


def _prep_host_small(inputs, kern):
    """Host-side input prep; inputs [Bn, 2048, 64] with Bn a multiple of GB."""
    import ml_dtypes

    bf = ml_dtypes.bfloat16
    Bn = inputs.shape[0]
    ng = Bn // GB
    X = np.ascontiguousarray(inputs, dtype=np.float32)
    W = np.ascontiguousarray(kern.reshape(IN_DIM, NUM * DIM), dtype=np.float32)

    # x[b, parity*64+G, kc, i] = X[b, (2kc+parity)*64+G, i]
    xr = X.reshape(Bn, 16, 2, 64, IN_DIM)          # [b, kc, parity, G, i]
    x_h = np.ascontiguousarray(xr.transpose(0, 2, 3, 1, 4).reshape(Bn, 128, 16, IN_DIM))
    xt = X.transpose(0, 2, 1)                      # [b, i, r]
    xt2_h = np.ascontiguousarray(np.concatenate([xt, xt], axis=1))  # [b, 128, 2048]
    # xs[grp, i, parity*64 + j*16 + kc] = sum_G X[b, (2kc+parity)*64+G, i]
    xsum = X.reshape(Bn, 16, 2, 64, IN_DIM).sum(axis=3)  # [b, kc, parity, i]
    xs_h = np.zeros((ng, IN_DIM, 128), np.float32)
    for grp in range(ng):
        for j in range(GB):
            for parity in range(2):
                blk = xsum[grp * GB + j, :, parity, :].T
                xs_h[grp, :, parity * 64 + j * 16 : parity * 64 + (j + 1) * 16] = blk
    w2_h = np.concatenate([W, W], axis=0)          # [128, 2048]
    # wt2c[d, g, dup*64+i] = W[i, g*64+d]
    wt = np.ascontiguousarray(W.reshape(IN_DIM, 32, 64).transpose(2, 1, 0))  # [d, g, i]
    wt2c_h = np.ascontiguousarray(np.concatenate([wt, wt], axis=2))          # [d, g, 128]
    wsum_h = np.ascontiguousarray(W.reshape(IN_DIM, 32, 64).sum(axis=1) / 32.0)
    i128_h = np.eye(128, dtype=np.float32)
    eye64 = np.eye(64, dtype=np.float32)
    ef_h = np.tile(eye64, (2, 2))                  # [128, 128]
    return (
        x_h.astype(bf), xt2_h.astype(bf), xs_h.astype(bf),
        w2_h.astype(bf), wt2c_h.astype(bf), wsum_h.astype(bf), i128_h.astype(bf),
        ef_h.astype(bf),
    )


def _make_in_maps(inputs, kern):
    x_h, xt2_h, xs_h, w2_h, wt2c_h, wsum_h, i128_h, ef_h = _prep_host_small(
        np.asarray(inputs), np.asarray(kern)
    )
    in_maps = []
    for c in range(N_CORES):
        sl = slice(c * BPC, (c + 1) * BPC)
        gsl = slice(c * NG, (c + 1) * NG)
        in_maps.append(
            {
                "x": x_h[sl], "xt2": xt2_h[sl], "xs": xs_h[gsl],
                "w2": w2_h, "wt2c": wt2c_h, "wsum": wsum_h, "i128": i128_h,
                "ef": ef_h,
            }
        )
    return in_maps


def kernel(inputs, kernel, num_capsule=NUM, dim_capsule=DIM, routings=3, **_):
    from concourse.bass_utils import run_bass_kernel_spmd

    assert int(num_capsule) == NUM and int(dim_capsule) == DIM and int(routings) == 3
    nc = _get_nc()
    in_maps = _make_in_maps(inputs, kernel)
    res = run_bass_kernel_spmd(nc, in_maps, core_ids=list(range(N_CORES)))
    out = np.concatenate([res.results[c]["out"] for c in range(N_CORES)], axis=0)
    return out.astype(np.float32)


# revision 13
# speedup vs baseline: 1.3123x; 1.3123x over previous
"""Capsule-routing kernel v3 — batch-merged, parity-packed, wave-pipelined.

Per core: 8 batches in 2 groups of 4, processed in lockstep so that matmuls
merge across batches and vector/scalar ops run at full [128, *] width.

Index conventions (per group of GB=4 batches):
  capsule n = 2*kc + parity   (kc in [0,16), parity in {0,1})
  slot(j, kc) = j*16 + kc     in [0, 64)
  p'(j, n)  = parity*64 + slot  -> o-all / oT / z column order  (parity-major)
  b/c layout: [128 (parity*64+G), 32 g, 64 slot]
Row-packed MM pairs: even capsules use partitions 0:64, odd 64:128, running
concurrently in distinct PE row groups (bf16 only).

v3 changes vs v2:
  - zstep: dup-column weights (wt2c [64 d, 32 g, 128 (dup,i)]) -> one N=128
    matmul per g writes both partition halves of z2; FWL-eligible 128-col LDW.
  - squash: f = exp(0.5*ln(s') - ln(1+s')) so the Act engine stays on the
    ln/exp/square table (no ACT_TABLE_LOAD thrash); f-scale fused into the
    PSUM->SBUF copy via activation(Copy, scale=f).
  - softmax/pstep wave-pipelined per batch j: exp/reduce/fold/recip/c-mul/
    pstep per 16-slot wave so the 2us exp is off the critical path.
  - bf16 expb/T/e2 (2x DVE reduce, 1-cycle fold MMs).
  - engine rebalance: PSUM->SBUF copies + b-adds spread over Vec/Scalar/Pool.
  - DMA: group-0 inputs first on sync queue, consts on gpsimd queue,
    one output DMA per group.
"""

import numpy as np

B, IN_CAPS, IN_DIM = 64, 2048, 64
NUM, DIM = 32, 64
N_CORES = 8
BPC = B // N_CORES  # 8 batches per core
GB = 4              # batches per merged group
NG = BPC // GB      # 2 groups
EPS = 1e-7

_CACHE = {}


def _build_nc(bpc=BPC):
    import concourse.bacc as bacc
    import concourse.tile as tile
    from concourse import mybir

    f32 = mybir.dt.float32
    bf16 = mybir.dt.bfloat16
    Act = mybir.ActivationFunctionType
    Alu = mybir.AluOpType

    ng = bpc // GB
    nc = bacc.Bacc("TRN2", target_bir_lowering=False, debug=False, num_devices=N_CORES)

    # ---- DRAM I/O (per-core shapes) ----
    # x[b, parity*64+G, kc, i] = X[b, (2kc+parity)*64+G, i]
    x_d = nc.dram_tensor("x", [bpc, 128, 16, IN_DIM], bf16, kind="ExternalInput")
    # xt2[b, q, r] = X[b, r, q % 64]   (stacked twice for row-packing)
    xt2_d = nc.dram_tensor("xt2", [bpc, 128, IN_CAPS], bf16, kind="ExternalInput")
    # xs[grp, i, p'] = sum_G X[b, n*64+G, i] at p' = parity*64 + j*16 + kc
    xs_d = nc.dram_tensor("xs", [ng, IN_DIM, 128], bf16, kind="ExternalInput")
    # w2 = W stacked twice: [128 (dup i), 2048]
    w2_d = nc.dram_tensor("w2", [128, NUM * DIM], bf16, kind="ExternalInput")
    # wt[d, g, i] = W[i, g*64+d]
    wt_d = nc.dram_tensor("wt", [IN_DIM, 32, IN_DIM], bf16, kind="ExternalInput")
    wsum_d = nc.dram_tensor("wsum", [IN_DIM, DIM], bf16, kind="ExternalInput")
    i128_d = nc.dram_tensor("i128", [128, 128], bf16, kind="ExternalInput")
    # ef[q, p] = 1 if q == p (mod 64): one matmul folds the parity halves of
    # T AND broadcasts the full softmax denominator to all 128 partitions
    ef_d = nc.dram_tensor("ef", [128, 128], bf16, kind="ExternalInput")
    out_d = nc.dram_tensor("out", [bpc, NUM, DIM], f32, kind="ExternalOutput")

    with tile.TileContext(nc) as tc:
        with (
            tc.tile_pool(name="const", bufs=1) as cpool,
            tc.tile_pool(name="inp", bufs=2) as ipool,
            tc.tile_pool(name="work", bufs=2) as wpool,
            tc.tile_pool(name="big", bufs=3) as bigpool,
            tc.tile_pool(name="ps_wave", bufs=2, space="PSUM") as ps_wave,
            tc.tile_pool(name="ps_z", bufs=2, space="PSUM") as ps_z,
            tc.tile_pool(name="ps_db", bufs=2, space="PSUM") as ps_db,
            tc.tile_pool(name="ps_o", bufs=1, space="PSUM") as ps_o,
            tc.tile_pool(name="ps_sm", bufs=1, space="PSUM") as ps_sm,
        ):
            # ---- input DMAs: group 0 first (sync queue), consts on gpsimd ----
            xs_t = [None] * ng
            xt2_t = [[None] * GB for _ in range(ng)]
            x_t = [[None] * GB for _ in range(ng)]
            wsum_t = cpool.tile([IN_DIM, DIM], bf16, tag="wsum")
            i128_t = cpool.tile([128, 128], bf16, tag="i128")
            wt_t = cpool.tile([IN_DIM, 32, IN_DIM], bf16, tag="wt")
            ef_t = cpool.tile([128, 128], bf16, tag="ef")
            w2_t = cpool.tile([128, NUM * DIM], bf16, tag="w2")

            def load_group(grp):
                for j in range(GB):
                    b = grp * GB + j
                    t = ipool.tile([128, IN_CAPS], bf16, tag=f"xt2_{grp}_{j}", name=f"xt2_{grp}_{j}")
                    nc.sync.dma_start(t[:], xt2_d[b])
                    xt2_t[grp][j] = t
                for j in range(GB):
                    b = grp * GB + j
                    t = ipool.tile([128, 16, IN_DIM], bf16, tag=f"x_{grp}_{j}", name=f"x_{grp}_{j}")
                    nc.sync.dma_start(t[:], x_d[b])
                    x_t[grp][j] = t

            nc.sync.dma_start(wsum_t[:], wsum_d[:])
            for grp in range(ng):
                xs_t[grp] = ipool.tile([IN_DIM, 128], bf16, tag=f"xs{grp}", name=f"xs{grp}")
                nc.sync.dma_start(xs_t[grp][:], xs_d[grp])
            load_group(0)
            # consts on the gpsimd queue so the sync queue stays on group data
            nc.gpsimd.dma_start(i128_t[:], i128_d[:])
            nc.gpsimd.dma_start(wt_t[:], wt_d[:])
            nc.gpsimd.dma_start(ef_t[:], ef_d[:])
            nc.gpsimd.dma_start(w2_t[:], w2_d[:])
            load_group(1)

            # pre-load the combined ln+exp+square act table (id 6,
            # natural_log_exp_and_others): the table-load pass then sees every
            # activation served and inserts no further ACT_TABLE_LOADs
            nc.scalar.add_instruction(mybir.InstLoadActFuncSet(
                name=nc.get_next_instruction_name(), ins=[], outs=[],
                act_func_set_id=6))

            eps_t = cpool.tile([128, 1], f32, tag="eps")
            nc.vector.memset(eps_t[:], EPS)
            one_eps_t = cpool.tile([128, 1], f32, tag="one_eps")
            nc.vector.memset(one_eps_t[:], 1.0 + EPS)

            # ---------------- squash ----------------
            def squash(o_ps, want_f32=False):
                """psum [128,64] -> (f32 sbuf or None, bf16 sbuf) squashed.

                f = sqrt(s')/(1+s') = exp(0.5*ln(s'+eps') - ln(1+s')); all Act
                ops stay in the ln/exp/square table set.
                """
                scr = wpool.tile([128, DIM], bf16, tag="scr")
                s0 = wpool.tile([128, 1], f32, tag="s0")
                nc.scalar.activation(scr[:], o_ps[:], Act.Square, accum_out=s0[:])
                la = wpool.tile([128, 1], f32, tag="la")
                nc.scalar.activation(la[:], s0[:], Act.Ln, bias=eps_t[:])
                lb = wpool.tile([128, 1], f32, tag="lb")
                nc.scalar.activation(lb[:], s0[:], Act.Ln, bias=one_eps_t[:])
                t = wpool.tile([128, 1], f32, tag="t")
                nc.vector.tensor_scalar(t[:], la[:], 0.5, lb[:], Alu.mult, Alu.subtract)
                f = wpool.tile([128, 1], f32, tag="f")
                nc.scalar.activation(f[:], t[:], Act.Exp)
                o_f32 = None
                if want_f32:
                    o_f32 = wpool.tile([128, DIM], f32, tag="osqf")
                    nc.scalar.activation(o_f32[:], o_ps[:], Act.Copy, scale=f[:])
                o_bf = wpool.tile([128, DIM], bf16, tag="osqb")
                nc.scalar.activation(o_bf[:], o_ps[:], Act.Copy, scale=f[:])
                return o_f32, o_bf

            def transpose_o(o_bf):
                """[128,64] bf16 -> oT sbuf [64,128] bf16."""
                t_ps = ps_wave.tile([IN_DIM, 128], bf16, tag="pw")
                nc.tensor.transpose(t_ps[:], o_bf[:], i128_t[:])
                oT = wpool.tile([IN_DIM, 128], bf16, tag="oT")
                nc.vector.tensor_copy(oT[:], t_ps[:])
                return oT

            def zstep(oT):
                """oT [64,128] -> z2 sbuf [128 (dup i), 32 g, 128 p'] bf16."""
                z2 = bigpool.tile([128, 32, 128], bf16, tag="z2")
                for gw in range(8):  # waves of 4 g
                    z_ps = ps_z.tile([128, 4, 128], f32, tag="z")
                    for j in range(4):
                        g = gw * 4 + j
                        nc.tensor.matmul(
                            z_ps[0:64, j, :], lhsT=wt_t[:, g, :], rhs=oT[:],
                            start=True, stop=True,
                        )
                        nc.tensor.matmul(
                            z_ps[64:128, j, :], lhsT=wt_t[:, g, :], rhs=oT[:],
                            start=True, stop=True,
                        )
                    dst = z2[:, gw * 4 : gw * 4 + 4, :]
                    if gw % 2 == 0:
                        nc.scalar.copy(dst, z_ps[:])
                    else:
                        nc.vector.tensor_copy(dst, z_ps[:])
                return z2

            def dbstep(z2, xt2_g, b_prev):
                """-> new b sbuf [128, 32 g, 64 slot] f32 (written per wave)."""
                nb = bigpool.tile([128, 32, 64], f32, tag="b")
                for j in range(GB):  # one wave per batch: 16 slots
                    db_ps = ps_db.tile([128, 32, 16], f32, tag="db")
                    for kc in range(16):
                        slot = j * 16 + kc
                        for parity in range(2):
                            h = parity * 64
                            n = 2 * kc + parity
                            nc.tensor.matmul(
                                db_ps[h : h + 64, :, kc],
                                lhsT=xt2_g[j][h : h + 64, n * 64 : (n + 1) * 64],
                                rhs=z2[h : h + 64, :, h + slot],
                                start=True, stop=True,
                            )
                    dst = nb[:, :, j * 16 : (j + 1) * 16]
                    if b_prev is None:
                        if j % 2 == 0:
                            nc.vector.tensor_copy(dst, db_ps[:])
                        else:
                            nc.scalar.copy(dst, db_ps[:])
                    else:
                        nc.vector.tensor_add(
                            dst, b_prev[:, :, j * 16 : (j + 1) * 16], db_ps[:]
                        )
                return nb

            def smp_step(b_sb, x_g):
                """softmax + pstep, wave-pipelined per batch j.

                b [128, 32 g, 64 slot] f32 -> p_all [128, 64 slot, 32 g] bf16.
                """
                expb = bigpool.tile([128, 32, 64], bf16, tag="expb")
                T = wpool.tile([128, GB, 32], bf16, tag="T")
                rs2 = wpool.tile([128, GB, 32], f32, tag="rs2")
                p_all = bigpool.tile([128, 64, 32], bf16, tag="pall")
                mul_eng = [nc.vector, nc.gpsimd, nc.vector, nc.gpsimd]
                for j in range(GB):
                    sl = slice(j * 16, (j + 1) * 16)
                    # exp of this batch's 16 slots (all g): [128, 32, 16]
                    nc.scalar.activation(expb[:, :, sl], b_sb[:, :, sl], Act.Exp)
                    # T[p, j, g] = sum_kc expb[p, g, j*16+kc]  (bf16 out: the
                    # 0.4% rounding on the softmax denominator is within budget
                    # and enables the 2x 16-bit DVE mode)
                    with nc.allow_low_precision(reason="softmax denom bf16"):
                        nc.vector.tensor_reduce(
                            T[:, j, :].unsqueeze(-1).squeeze(-1),
                            expb[:, :, sl],
                            mybir.AxisListType.X, Alu.add,
                        )
                    # S2[p, g] = sum over both parity halves, broadcast to
                    # all 128 partitions in one matmul
                    S2_ps = ps_sm.tile([128, 32], f32, tag="sm")
                    nc.tensor.matmul(
                        S2_ps[:], lhsT=ef_t[:], rhs=T[:, j, :],
                        start=True, stop=True,
                    )
                    nc.vector.reciprocal(rs2[:, j, :], S2_ps[:])
                    # c = expb * 1/S2 (broadcast over kc), in place
                    mul_eng[j].tensor_mul(
                        expb[:, :, sl],
                        expb[:, :, sl],
                        rs2[:, j, :, None].to_broadcast([128, 32, 16]),
                    )
                    # pstep for this batch's 16 slots
                    p_ps = ps_wave.tile([128, 16, 32], f32, tag="pw")
                    for kc in range(16):
                        slot = j * 16 + kc
                        for parity in range(2):
                            h = parity * 64
                            nc.tensor.matmul(
                                p_ps[h : h + 64, kc, :],
                                lhsT=x_g[j][h : h + 64, kc, :],
                                rhs=expb[h : h + 64, :, slot],
                                start=True, stop=True,
                            )
                    if j % 2 == 0:
                        nc.scalar.copy(p_all[:, sl, :], p_ps[:])
                    else:
                        nc.vector.tensor_copy(p_all[:, sl, :], p_ps[:])
                return p_all

            def mm2(p_all, o_ps):
                # MM2: per g, row-packed even/odd halves accumulate into o-all.
                for g in range(32):
                    nc.tensor.matmul(
                        o_ps[0:64, :],
                        lhsT=p_all[0:64, :, g],
                        rhs=w2_t[0:64, g * 64 : (g + 1) * 64],
                        start=(g == 0), stop=(g == 31),
                        skip_group_check=True,
                    )
                    nc.tensor.matmul(
                        o_ps[64:128, :],
                        lhsT=p_all[64:128, :, g],
                        rhs=w2_t[64:128, g * 64 : (g + 1) * 64],
                        start=(g == 0), stop=(g == 31),
                        skip_group_check=True,
                    )

            # ================= interleaved group emission =================
            st = [dict() for _ in range(ng)]

            def ph_iter0(g_):
                s_ = st[g_]
                o_ps = ps_o.tile([128, DIM], f32, tag="o")
                nc.tensor.matmul(o_ps[:], lhsT=xs_t[g_][:], rhs=wsum_t[:], start=True, stop=True)
                s_["o_f32"], s_["o_bf"] = squash(o_ps)
                s_["b"] = None
                s_["m2s_count"] = 0

            def ph_tz(g_):
                s_ = st[g_]
                oT = transpose_o(s_["o_bf"])
                s_["z2"] = zstep(oT)

            def ph_db(g_):
                s_ = st[g_]
                s_["b"] = dbstep(s_["z2"], xt2_t[g_], s_["b"])

            def ph_smp(g_):
                s_ = st[g_]
                s_["pall"] = smp_step(s_["b"], x_t[g_])

            def ph_m2s(g_):
                s_ = st[g_]
                o_ps = ps_o.tile([128, DIM], f32, tag="o")
                mm2(s_["pall"], o_ps)
                s_["m2s_count"] += 1
                s_["o_f32"], s_["o_bf"] = squash(o_ps, want_f32=(s_["m2s_count"] == 2))

            def ph_out(g_):
                grp, s_ = g_, st[g_]
                # one DMA per group: sbuf [128 (par, j, kc), 64] -> dram view
                dst = out_d[grp * GB : (grp + 1) * GB].rearrange(
                    "b (kc par) d -> par b kc d", par=2
                )
                nc.gpsimd.dma_start(dst, s_["o_f32"][:])

            phases = [ph_iter0, ph_tz, ph_db, ph_smp, ph_m2s,
                      ph_tz, ph_db, ph_smp, ph_m2s, ph_out]
            OFFSET = 1
            for k in range(len(phases) + OFFSET * (ng - 1)):
                for grp in range(ng):
                    kk = k - OFFSET * grp
                    if 0 <= kk < len(phases):
                        phases[kk](grp)

    nc.compile()
    return nc


def _get_nc():
    if "nc" not in _CACHE:
        _CACHE["nc"] = _build_nc()
    return _CACHE["nc"]


def _prep_host_small(inputs, kern):
    """Host-side input prep; inputs [Bn, 2048, 64] with Bn a multiple of GB."""
    import ml_dtypes

    bf = ml_dtypes.bfloat16
    Bn = inputs.shape[0]
    ng = Bn // GB
    X = np.ascontiguousarray(inputs, dtype=np.float32)
    W = np.ascontiguousarray(kern.reshape(IN_DIM, NUM * DIM), dtype=np.float32)

    # x[b, parity*64+G, kc, i] = X[b, (2kc+parity)*64+G, i]
    xr = X.reshape(Bn, 16, 2, 64, IN_DIM)          # [b, kc, parity, G, i]
    x_h = np.ascontiguousarray(xr.transpose(0, 2, 3, 1, 4).reshape(Bn, 128, 16, IN_DIM))
    xt = X.transpose(0, 2, 1)                      # [b, i, r]
    xt2_h = np.ascontiguousarray(np.concatenate([xt, xt], axis=1))  # [b, 128, 2048]
    # xs[grp, i, parity*64 + j*16 + kc] = sum_G X[b, (2kc+parity)*64+G, i]
    xsum = X.reshape(Bn, 16, 2, 64, IN_DIM).sum(axis=3)  # [b, kc, parity, i]
    xs_h = np.zeros((ng, IN_DIM, 128), np.float32)
    for grp in range(ng):
        for j in range(GB):
            for parity in range(2):
                blk = xsum[grp * GB + j, :, parity, :].T
                xs_h[grp, :, parity * 64 + j * 16 : parity * 64 + (j + 1) * 16] = blk
    w2_h = np.concatenate([W, W], axis=0)          # [128, 2048]
    wt_h = np.ascontiguousarray(W.reshape(IN_DIM, 32, 64).transpose(2, 1, 0))  # [d, g, i]
    wsum_h = np.ascontiguousarray(W.reshape(IN_DIM, 32, 64).sum(axis=1) / 32.0)
    i128_h = np.eye(128, dtype=np.float32)
    eye64 = np.eye(64, dtype=np.float32)
    ef_h = np.tile(eye64, (2, 2))                  # [128, 128]
    return (
        x_h.astype(bf), xt2_h.astype(bf), xs_h.astype(bf),
        w2_h.astype(bf), wt_h.astype(bf), wsum_h.astype(bf), i128_h.astype(bf),
        ef_h.astype(bf),
    )


def _make_in_maps(inputs, kern):
    x_h, xt2_h, xs_h, w2_h, wt_h, wsum_h, i128_h, ef_h = _prep_host_small(
        np.asarray(inputs), np.asarray(kern)
    )
    in_maps = []
    for c in range(N_CORES):
        sl = slice(c * BPC, (c + 1) * BPC)
        gsl = slice(c * NG, (c + 1) * NG)
        in_maps.append(
            {
                "x": x_h[sl], "xt2": xt2_h[sl], "xs": xs_h[gsl],
                "w2": w2_h, "wt": wt_h, "wsum": wsum_h, "i128": i128_h,
                "ef": ef_h,
            }
        )
    return in_maps


def kernel(inputs, kernel, num_capsule=NUM, dim_capsule=DIM, routings=3, **_):
    from concourse.bass_utils import run_bass_kernel_spmd

    assert int(num_capsule) == NUM and int(dim_capsule) == DIM and int(routings) == 3
    nc = _get_nc()
    in_maps = _make_in_maps(inputs, kernel)
    res = run_bass_kernel_spmd(nc, in_maps, core_ids=list(range(N_CORES)))
    out = np.concatenate([res.results[c]["out"] for c in range(N_CORES)], axis=0)
    return out.astype(np.float32)


# revision 16
# speedup vs baseline: 1.3806x; 1.0520x over previous
"""Capsule-routing kernel v3 — batch-merged, parity-packed, wave-pipelined.

Per core: 8 batches in 2 groups of 4, processed in lockstep so that matmuls
merge across batches and vector/scalar ops run at full [128, *] width.

Index conventions (per group of GB=4 batches):
  capsule n = 2*kc + parity   (kc in [0,16), parity in {0,1})
  slot(j, kc) = j*16 + kc     in [0, 64)
  p'(j, n)  = parity*64 + slot  -> o-all / oT / z column order  (parity-major)
  b/c layout: [128 (parity*64+G), 32 g, 64 slot]
Row-packed MM pairs: even capsules use partitions 0:64, odd 64:128, running
concurrently in distinct PE row groups (bf16 only).

v3 changes vs v2:
  - zstep: dup-column weights (wt2c [64 d, 32 g, 128 (dup,i)]) -> one N=128
    matmul per g writes both partition halves of z2; FWL-eligible 128-col LDW.
  - squash: f = exp(0.5*ln(s') - ln(1+s')) so the Act engine stays on the
    ln/exp/square table (no ACT_TABLE_LOAD thrash); f-scale fused into the
    PSUM->SBUF copy via activation(Copy, scale=f).
  - softmax/pstep wave-pipelined per batch j: exp/reduce/fold/recip/c-mul/
    pstep per 16-slot wave so the 2us exp is off the critical path.
  - bf16 expb/T/e2 (2x DVE reduce, 1-cycle fold MMs).
  - engine rebalance: PSUM->SBUF copies + b-adds spread over Vec/Scalar/Pool.
  - DMA: group-0 inputs first on sync queue, consts on gpsimd queue,
    one output DMA per group.
"""

import numpy as np

B, IN_CAPS, IN_DIM = 64, 2048, 64
NUM, DIM = 32, 64
N_CORES = 8
BPC = B // N_CORES  # 8 batches per core
GB = 4              # batches per merged group
NG = BPC // GB      # 2 groups
EPS = 1e-7

_CACHE = {}


def _build_nc(bpc=BPC):
    import concourse.bacc as bacc
    import concourse.tile as tile
    from concourse import mybir

    f32 = mybir.dt.float32
    bf16 = mybir.dt.bfloat16
    Act = mybir.ActivationFunctionType
    Alu = mybir.AluOpType

    ng = bpc // GB
    nc = bacc.Bacc("TRN2", target_bir_lowering=False, debug=False, num_devices=N_CORES)

    # ---- DRAM I/O (per-core shapes) ----
    # x[b, parity*64+G, kc, i] = X[b, (2kc+parity)*64+G, i]
    x_d = nc.dram_tensor("x", [bpc, 128, 16, IN_DIM], bf16, kind="ExternalInput")
    # xt2[b, q, r] = X[b, r, q % 64]   (stacked twice for row-packing)
    xt2_d = nc.dram_tensor("xt2", [bpc, 128, IN_CAPS], bf16, kind="ExternalInput")
    # xs[grp, i, p'] = sum_G X[b, n*64+G, i] at p' = parity*64 + j*16 + kc
    xs_d = nc.dram_tensor("xs", [ng, IN_DIM, 128], bf16, kind="ExternalInput")
    # w2 = W stacked twice: [128 (dup i), 2048]
    w2_d = nc.dram_tensor("w2", [128, NUM * DIM], bf16, kind="ExternalInput")
    # wt[d, g, i] = W[i, g*64+d]
    wt_d = nc.dram_tensor("wt", [IN_DIM, 32, IN_DIM], bf16, kind="ExternalInput")
    wsum_d = nc.dram_tensor("wsum", [IN_DIM, DIM], bf16, kind="ExternalInput")
    i128_d = nc.dram_tensor("i128", [128, 128], bf16, kind="ExternalInput")
    # ef[q, p] = 1 if q == p (mod 64): one matmul folds the parity halves of
    # T AND broadcasts the full softmax denominator to all 128 partitions
    ef_d = nc.dram_tensor("ef", [128, 128], bf16, kind="ExternalInput")
    out_d = nc.dram_tensor("out", [bpc, NUM, DIM], f32, kind="ExternalOutput")

    with tile.TileContext(nc) as tc:
        with (
            tc.tile_pool(name="const", bufs=1) as cpool,
            tc.tile_pool(name="inp", bufs=2) as ipool,
            tc.tile_pool(name="work", bufs=2) as wpool,
            tc.tile_pool(name="big", bufs=3) as bigpool,
            tc.tile_pool(name="ps_wave", bufs=2, space="PSUM") as ps_wave,
            tc.tile_pool(name="ps_z", bufs=2, space="PSUM") as ps_z,
            tc.tile_pool(name="ps_db", bufs=2, space="PSUM") as ps_db,
            tc.tile_pool(name="ps_o", bufs=1, space="PSUM") as ps_o,
            tc.tile_pool(name="ps_sm", bufs=1, space="PSUM") as ps_sm,
        ):
            # ---- input DMAs: group 0 first (sync queue), consts on gpsimd ----
            xs_t = [None] * ng
            xt2_t = [[None] * GB for _ in range(ng)]
            x_t = [[None] * GB for _ in range(ng)]
            wsum_t = cpool.tile([IN_DIM, DIM], bf16, tag="wsum")
            i128_t = cpool.tile([128, 128], bf16, tag="i128")
            wt_t = cpool.tile([IN_DIM, 32, IN_DIM], bf16, tag="wt")
            ef_t = cpool.tile([128, 128], bf16, tag="ef")
            w2_t = cpool.tile([128, NUM * DIM], bf16, tag="w2")

            def load_group(grp):
                for j in range(GB):
                    b = grp * GB + j
                    t = ipool.tile([128, IN_CAPS], bf16, tag=f"xt2_{grp}_{j}", name=f"xt2_{grp}_{j}")
                    nc.sync.dma_start(t[:], xt2_d[b])
                    xt2_t[grp][j] = t
                for j in range(GB):
                    b = grp * GB + j
                    t = ipool.tile([128, 16, IN_DIM], bf16, tag=f"x_{grp}_{j}", name=f"x_{grp}_{j}")
                    nc.sync.dma_start(t[:], x_d[b])
                    x_t[grp][j] = t

            nc.sync.dma_start(wsum_t[:], wsum_d[:])
            for grp in range(ng):
                xs_t[grp] = ipool.tile([IN_DIM, 128], bf16, tag=f"xs{grp}", name=f"xs{grp}")
                nc.sync.dma_start(xs_t[grp][:], xs_d[grp])
            load_group(0)
            # consts on the gpsimd queue so the sync queue stays on group data
            nc.gpsimd.dma_start(i128_t[:], i128_d[:])
            nc.gpsimd.dma_start(wt_t[:], wt_d[:])
            nc.gpsimd.dma_start(ef_t[:], ef_d[:])
            nc.gpsimd.dma_start(w2_t[:], w2_d[:])
            load_group(1)

            # pre-load the combined ln+exp+square act table (id 6,
            # natural_log_exp_and_others): the table-load pass then sees every
            # activation served and inserts no further ACT_TABLE_LOADs
            nc.scalar.add_instruction(mybir.InstLoadActFuncSet(
                name=nc.get_next_instruction_name(), ins=[], outs=[],
                act_func_set_id=6))

            eps_t = cpool.tile([128, 1], f32, tag="eps")
            nc.vector.memset(eps_t[:], EPS)
            # PE warmup: ~3.4us of dummy matmuls during the input-DMA wait so
            # the HAM clock gate reaches 8/8 (2.4 GHz) before real work starts
            warm_t = cpool.tile([IN_DIM, 128], bf16, tag="warm")
            nc.vector.memset(warm_t[:], 0.0)
            wz_ps = ps_z.tile([128, 4, 128], f32, tag="z")
            for wi in range(16):
                nc.tensor.matmul(
                    wz_ps[:, wi % 4, :], lhsT=warm_t[:], rhs=warm_t[:],
                    start=True, stop=True, skip_group_check=True,
                )
            one_eps_t = cpool.tile([128, 1], f32, tag="one_eps")
            nc.vector.memset(one_eps_t[:], 1.0 + EPS)

            # ---------------- squash ----------------
            def squash(o_ps, want_f32=False, for_transpose=True):
                """psum [128,64] -> (o_f32|None, o_raw bf16|None, diag(f)|None).

                f = sqrt(s')/(1+s') with u = exp(0.5*ln(s')) on Act while the
                1/(1+s') branch runs on DVE in parallel; all Act funcs live in
                the ln/exp/square table.  f is applied to the routing copy via
                diag(f) as the transpose identity (never on the o_raw copy,
                which starts right after mm2).
                """
                scr = wpool.tile([128, DIM], bf16, tag="scr")
                s0 = wpool.tile([128, 1], f32, tag="s0")
                nc.scalar.activation(scr[:], o_ps[:], Act.Square, accum_out=s0[:])
                o_raw = None
                if for_transpose:
                    o_raw = wpool.tile([128, DIM], bf16, tag="osqb")
                    nc.vector.tensor_copy(o_raw[:], o_ps[:])
                la = wpool.tile([128, 1], f32, tag="la")
                nc.scalar.activation(la[:], s0[:], Act.Ln, bias=eps_t[:])
                u = wpool.tile([128, 1], f32, tag="u")
                nc.scalar.activation(u[:], la[:], Act.Exp, scale=0.5)
                v = wpool.tile([128, 1], f32, tag="v")
                nc.vector.tensor_scalar_add(v[:], s0[:], 1.0 + EPS)
                rv = wpool.tile([128, 1], f32, tag="rv")
                nc.vector.reciprocal(rv[:], v[:])
                f = wpool.tile([128, 1], f32, tag="f")
                nc.vector.tensor_mul(f[:], u[:], rv[:])
                o_f32 = None
                if want_f32:
                    o_f32 = wpool.tile([128, DIM], f32, tag="osqf")
                    nc.scalar.activation(o_f32[:], o_ps[:], Act.Copy, scale=f[:])
                df = None
                if for_transpose:
                    df = wpool.tile([128, 128], bf16, tag="df")
                    nc.vector.tensor_scalar_mul(df[:], i128_t[:], f[:])
                return o_f32, o_raw, df

            def transpose_o(o_raw, df):
                """oT = (f*o)^T via a normal matmul: out[d, q] =
                sum_p o_raw[p, d] * diag(f)[p, q] = o[q, d] * f[q]."""
                t_ps = ps_wave.tile([IN_DIM, 128], f32, tag="pw")
                nc.tensor.matmul(t_ps[:], lhsT=o_raw[:], rhs=df[:],
                                 start=True, stop=True)
                oT = wpool.tile([IN_DIM, 128], bf16, tag="oT")
                nc.vector.tensor_copy(oT[:], t_ps[:])
                return oT

            def zstep(oT):
                """oT [64,128] -> z2 sbuf [128 (dup i), 32 g, 128 p'] bf16."""
                z2 = bigpool.tile([128, 32, 128], bf16, tag="z2")
                for gw in range(8):  # waves of 4 g
                    z_ps = ps_z.tile([128, 4, 128], f32, tag="z")
                    for j in range(4):
                        g = gw * 4 + j
                        nc.tensor.matmul(
                            z_ps[0:64, j, :], lhsT=wt_t[:, g, :], rhs=oT[:],
                            start=True, stop=True,
                        )
                        nc.tensor.matmul(
                            z_ps[64:128, j, :], lhsT=wt_t[:, g, :], rhs=oT[:],
                            start=True, stop=True,
                        )
                    dst = z2[:, gw * 4 : gw * 4 + 4, :]
                    if gw % 2 == 0:
                        nc.scalar.copy(dst, z_ps[:])
                    else:
                        nc.vector.tensor_copy(dst, z_ps[:])
                return z2

            def dbstep(z2, xt2_g, b_prev):
                """-> new b sbuf [128, 32 g, 64 slot] f32 (written per wave)."""
                nb = bigpool.tile([128, 32, 64], f32, tag="b")
                for j in range(GB):  # one wave per batch: 16 slots
                    db_ps = ps_db.tile([128, 32, 16], f32, tag="db")
                    for kc in range(16):
                        slot = j * 16 + kc
                        for parity in range(2):
                            h = parity * 64
                            n = 2 * kc + parity
                            nc.tensor.matmul(
                                db_ps[h : h + 64, :, kc],
                                lhsT=xt2_g[j][h : h + 64, n * 64 : (n + 1) * 64],
                                rhs=z2[h : h + 64, :, h + slot],
                                start=True, stop=True,
                            )
                    dst = nb[:, :, j * 16 : (j + 1) * 16]
                    if b_prev is None:
                        if j % 2 == 0:
                            nc.vector.tensor_copy(dst, db_ps[:])
                        else:
                            nc.scalar.copy(dst, db_ps[:])
                    else:
                        nc.vector.tensor_add(
                            dst, b_prev[:, :, j * 16 : (j + 1) * 16], db_ps[:]
                        )
                return nb

            def smp_step(b_sb, x_g):
                """softmax + pstep, wave-pipelined per batch j.

                b [128, 32 g, 64 slot] f32 -> p_all [128, 64 slot, 32 g] bf16.
                """
                expb = bigpool.tile([128, 32, 64], bf16, tag="expb")
                T = wpool.tile([128, GB, 32], bf16, tag="T")
                rs2 = wpool.tile([128, GB, 32], f32, tag="rs2")
                p_all = bigpool.tile([128, 32, 64], bf16, tag="pall")
                mul_eng = [nc.vector, nc.gpsimd, nc.vector, nc.gpsimd]
                for j in range(GB):
                    sl = slice(j * 16, (j + 1) * 16)
                    # exp of this batch's 16 slots (all g): [128, 32, 16]
                    nc.scalar.activation(expb[:, :, sl], b_sb[:, :, sl], Act.Exp)
                    # T[p, j, g] = sum_kc expb[p, g, j*16+kc]  (bf16 out: the
                    # 0.4% rounding on the softmax denominator is within budget
                    # and enables the 2x 16-bit DVE mode)
                    with nc.allow_low_precision(reason="softmax denom bf16"):
                        nc.vector.tensor_reduce(
                            T[:, j, :].unsqueeze(-1).squeeze(-1),
                            expb[:, :, sl],
                            mybir.AxisListType.X, Alu.add,
                        )
                    # S2[p, g] = sum over both parity halves, broadcast to
                    # all 128 partitions in one matmul
                    S2_ps = ps_sm.tile([128, 32], f32, tag="sm")
                    nc.tensor.matmul(
                        S2_ps[:], lhsT=ef_t[:], rhs=T[:, j, :],
                        start=True, stop=True,
                    )
                    nc.vector.reciprocal(rs2[:, j, :], S2_ps[:])
                    # c = expb * 1/S2 (broadcast over kc), in place
                    mul_eng[j].tensor_mul(
                        expb[:, :, sl],
                        expb[:, :, sl],
                        rs2[:, j, :, None].to_broadcast([128, 32, 16]),
                    )
                    # pstep for this batch's 16 slots (g-major psum so the
                    # mm2 weight slices come out contiguous)
                    p_ps = ps_wave.tile([128, 32, 16], f32, tag="pw")
                    for kc in range(16):
                        slot = j * 16 + kc
                        for parity in range(2):
                            h = parity * 64
                            nc.tensor.matmul(
                                p_ps[h : h + 64, :, kc],
                                lhsT=x_g[j][h : h + 64, kc, :],
                                rhs=expb[h : h + 64, :, slot],
                                start=True, stop=True,
                            )
                    if j % 2 == 0:
                        nc.scalar.copy(p_all[:, :, sl], p_ps[:])
                    else:
                        nc.vector.tensor_copy(p_all[:, :, sl], p_ps[:])
                return p_all

            def mm2(p_all, o_ps):
                # MM2: per g, row-packed even/odd halves accumulate into o-all.
                for g in range(32):
                    nc.tensor.matmul(
                        o_ps[0:64, :],
                        lhsT=p_all[0:64, g, :],
                        rhs=w2_t[0:64, g * 64 : (g + 1) * 64],
                        start=(g == 0), stop=(g == 31),
                        skip_group_check=True,
                    )
                    nc.tensor.matmul(
                        o_ps[64:128, :],
                        lhsT=p_all[64:128, g, :],
                        rhs=w2_t[64:128, g * 64 : (g + 1) * 64],
                        start=(g == 0), stop=(g == 31),
                        skip_group_check=True,
                    )

            # ================= interleaved group emission =================
            st = [dict() for _ in range(ng)]

            def ph_iter0(g_):
                s_ = st[g_]
                o_ps = ps_o.tile([128, DIM], f32, tag="o")
                nc.tensor.matmul(o_ps[:], lhsT=xs_t[g_][:], rhs=wsum_t[:], start=True, stop=True)
                _, s_["o_raw"], s_["df"] = squash(o_ps)
                s_["b"] = None
                s_["m2s_count"] = 0

            def ph_tz(g_):
                s_ = st[g_]
                oT = transpose_o(s_["o_raw"], s_["df"])
                s_["z2"] = zstep(oT)

            def ph_db(g_):
                s_ = st[g_]
                s_["b"] = dbstep(s_["z2"], xt2_t[g_], s_["b"])

            def ph_smp(g_):
                s_ = st[g_]
                s_["pall"] = smp_step(s_["b"], x_t[g_])

            def ph_m2s(g_):
                s_ = st[g_]
                o_ps = ps_o.tile([128, DIM], f32, tag="o")
                mm2(s_["pall"], o_ps)
                s_["m2s_count"] += 1
                final = s_["m2s_count"] == 2
                s_["o_f32"], s_["o_raw"], s_["df"] = squash(
                    o_ps, want_f32=final, for_transpose=not final)

            def ph_out(g_):
                grp, s_ = g_, st[g_]
                # one DMA per group: sbuf [128 (par, j, kc), 64] -> dram view
                dst = out_d[grp * GB : (grp + 1) * GB].rearrange(
                    "b (kc par) d -> par b kc d", par=2
                )
                nc.gpsimd.dma_start(dst, s_["o_f32"][:])

            phases = [ph_iter0, ph_tz, ph_db, ph_smp, ph_m2s,
                      ph_tz, ph_db, ph_smp, ph_m2s, ph_out]
            OFFSET = 1
            for k in range(len(phases) + OFFSET * (ng - 1)):
                for grp in range(ng):
                    kk = k - OFFSET * grp
                    if 0 <= kk < len(phases):
                        phases[kk](grp)

    nc.compile()
    return nc


def _get_nc():
    if "nc" not in _CACHE:
        _CACHE["nc"] = _build_nc()
    return _CACHE["nc"]


def _prep_host_small(inputs, kern):
    """Host-side input prep; inputs [Bn, 2048, 64] with Bn a multiple of GB."""
    import ml_dtypes

    bf = ml_dtypes.bfloat16
    Bn = inputs.shape[0]
    ng = Bn // GB
    X = np.ascontiguousarray(inputs, dtype=np.float32)
    W = np.ascontiguousarray(kern.reshape(IN_DIM, NUM * DIM), dtype=np.float32)

    # x[b, parity*64+G, kc, i] = X[b, (2kc+parity)*64+G, i]
    xr = X.reshape(Bn, 16, 2, 64, IN_DIM)          # [b, kc, parity, G, i]
    x_h = np.ascontiguousarray(xr.transpose(0, 2, 3, 1, 4).reshape(Bn, 128, 16, IN_DIM))
    xt = X.transpose(0, 2, 1)                      # [b, i, r]
    xt2_h = np.ascontiguousarray(np.concatenate([xt, xt], axis=1))  # [b, 128, 2048]
    # xs[grp, i, parity*64 + j*16 + kc] = sum_G X[b, (2kc+parity)*64+G, i]
    xsum = X.reshape(Bn, 16, 2, 64, IN_DIM).sum(axis=3)  # [b, kc, parity, i]
    xs_h = np.zeros((ng, IN_DIM, 128), np.float32)
    for grp in range(ng):
        for j in range(GB):
            for parity in range(2):
                blk = xsum[grp * GB + j, :, parity, :].T
                xs_h[grp, :, parity * 64 + j * 16 : parity * 64 + (j + 1) * 16] = blk
    w2_h = np.concatenate([W, W], axis=0)          # [128, 2048]
    wt_h = np.ascontiguousarray(W.reshape(IN_DIM, 32, 64).transpose(2, 1, 0))  # [d, g, i]
    wsum_h = np.ascontiguousarray(W.reshape(IN_DIM, 32, 64).sum(axis=1) / 32.0)
    i128_h = np.eye(128, dtype=np.float32)
    eye64 = np.eye(64, dtype=np.float32)
    ef_h = np.tile(eye64, (2, 2))                  # [128, 128]
    return (
        x_h.astype(bf), xt2_h.astype(bf), xs_h.astype(bf),
        w2_h.astype(bf), wt_h.astype(bf), wsum_h.astype(bf), i128_h.astype(bf),
        ef_h.astype(bf),
    )


def _make_in_maps(inputs, kern):
    x_h, xt2_h, xs_h, w2_h, wt_h, wsum_h, i128_h, ef_h = _prep_host_small(
        np.asarray(inputs), np.asarray(kern)
    )
    in_maps = []
    for c in range(N_CORES):
        sl = slice(c * BPC, (c + 1) * BPC)
        gsl = slice(c * NG, (c + 1) * NG)
        in_maps.append(
            {
                "x": x_h[sl], "xt2": xt2_h[sl], "xs": xs_h[gsl],
                "w2": w2_h, "wt": wt_h, "wsum": wsum_h, "i128": i128_h,
                "ef": ef_h,
            }
        )
    return in_maps


def kernel(inputs, kernel, num_capsule=NUM, dim_capsule=DIM, routings=3, **_):
    from concourse.bass_utils import run_bass_kernel_spmd

    assert int(num_capsule) == NUM and int(dim_capsule) == DIM and int(routings) == 3
    nc = _get_nc()
    in_maps = _make_in_maps(inputs, kernel)
    res = run_bass_kernel_spmd(nc, in_maps, core_ids=list(range(N_CORES)))
    out = np.concatenate([res.results[c]["out"] for c in range(N_CORES)], axis=0)
    return out.astype(np.float32)


# revision 17
# speedup vs baseline: 1.4048x; 1.0175x over previous
"""Capsule-routing kernel v3 — batch-merged, parity-packed, wave-pipelined.

Per core: 8 batches in 2 groups of 4, processed in lockstep so that matmuls
merge across batches and vector/scalar ops run at full [128, *] width.

Index conventions (per group of GB=4 batches):
  capsule n = 2*kc + parity   (kc in [0,16), parity in {0,1})
  slot(j, kc) = j*16 + kc     in [0, 64)
  p'(j, n)  = parity*64 + slot  -> o-all / oT / z column order  (parity-major)
  b/c layout: [128 (parity*64+G), 32 g, 64 slot]
Row-packed MM pairs: even capsules use partitions 0:64, odd 64:128, running
concurrently in distinct PE row groups (bf16 only).

v3 changes vs v2:
  - zstep: dup-column weights (wt2c [64 d, 32 g, 128 (dup,i)]) -> one N=128
    matmul per g writes both partition halves of z2; FWL-eligible 128-col LDW.
  - squash: f = exp(0.5*ln(s') - ln(1+s')) so the Act engine stays on the
    ln/exp/square table (no ACT_TABLE_LOAD thrash); f-scale fused into the
    PSUM->SBUF copy via activation(Copy, scale=f).
  - softmax/pstep wave-pipelined per batch j: exp/reduce/fold/recip/c-mul/
    pstep per 16-slot wave so the 2us exp is off the critical path.
  - bf16 expb/T/e2 (2x DVE reduce, 1-cycle fold MMs).
  - engine rebalance: PSUM->SBUF copies + b-adds spread over Vec/Scalar/Pool.
  - DMA: group-0 inputs first on sync queue, consts on gpsimd queue,
    one output DMA per group.
"""

import numpy as np

B, IN_CAPS, IN_DIM = 64, 2048, 64
NUM, DIM = 32, 64
N_CORES = 8
BPC = B // N_CORES  # 8 batches per core
GB = 4              # batches per merged group
NG = BPC // GB      # 2 groups
EPS = 1e-7

_CACHE = {}


def _build_nc(bpc=BPC):
    import concourse.bacc as bacc
    import concourse.tile as tile
    from concourse import mybir

    f32 = mybir.dt.float32
    bf16 = mybir.dt.bfloat16
    Act = mybir.ActivationFunctionType
    Alu = mybir.AluOpType

    ng = bpc // GB
    nc = bacc.Bacc("TRN2", target_bir_lowering=False, debug=False, num_devices=N_CORES)

    # ---- DRAM I/O (per-core shapes) ----
    # x[b, parity*64+G, kc, i] = X[b, (2kc+parity)*64+G, i]
    x_d = nc.dram_tensor("x", [bpc, 128, 16, IN_DIM], bf16, kind="ExternalInput")
    # xt[b, i, r] = X[b, r, i]
    xt2_d = nc.dram_tensor("xt2", [bpc, IN_DIM, IN_CAPS], bf16, kind="ExternalInput")
    # xs[grp, i, p'] = sum_G X[b, n*64+G, i] at p' = parity*64 + j*16 + kc
    xs_d = nc.dram_tensor("xs", [ng, IN_DIM, 128], bf16, kind="ExternalInput")
    # w2 = W stacked twice: [128 (dup i), 2048]
    w2_d = nc.dram_tensor("w2", [128, NUM * DIM], bf16, kind="ExternalInput")
    # wt[d, g, i] = W[i, g*64+d]
    wt_d = nc.dram_tensor("wt", [IN_DIM, 32, IN_DIM], bf16, kind="ExternalInput")
    wsum_d = nc.dram_tensor("wsum", [IN_DIM, DIM], bf16, kind="ExternalInput")
    i128_d = nc.dram_tensor("i128", [128, 128], bf16, kind="ExternalInput")
    # ef[q, p] = 1 if q == p (mod 64): one matmul folds the parity halves of
    # T AND broadcasts the full softmax denominator to all 128 partitions
    ef_d = nc.dram_tensor("ef", [128, 128], bf16, kind="ExternalInput")
    out_d = nc.dram_tensor("out", [bpc, NUM, DIM], f32, kind="ExternalOutput")

    with tile.TileContext(nc) as tc:
        with (
            tc.tile_pool(name="const", bufs=1) as cpool,
            tc.tile_pool(name="inp", bufs=2) as ipool,
            tc.tile_pool(name="work", bufs=2) as wpool,
            tc.tile_pool(name="big", bufs=3) as bigpool,
            tc.tile_pool(name="ps_wave", bufs=2, space="PSUM") as ps_wave,
            tc.tile_pool(name="ps_z", bufs=2, space="PSUM") as ps_z,
            tc.tile_pool(name="ps_db", bufs=2, space="PSUM") as ps_db,
            tc.tile_pool(name="ps_o", bufs=1, space="PSUM") as ps_o,
            tc.tile_pool(name="ps_sm", bufs=1, space="PSUM") as ps_sm,
        ):
            # ---- input DMAs: group 0 first (sync queue), consts on gpsimd ----
            xs_t = [None] * ng
            xt2_t = [[None] * GB for _ in range(ng)]
            x_t = [[None] * GB for _ in range(ng)]
            wsum_t = cpool.tile([IN_DIM, DIM], bf16, tag="wsum")
            i128_t = cpool.tile([128, 128], bf16, tag="i128")
            wt_t = cpool.tile([IN_DIM, 32, IN_DIM], bf16, tag="wt")
            ef_t = cpool.tile([128, 128], bf16, tag="ef")
            w2_t = cpool.tile([128, NUM * DIM], bf16, tag="w2")

            def load_group(grp):
                for j in range(GB):
                    b = grp * GB + j
                    t = ipool.tile([IN_DIM, IN_CAPS], bf16, tag=f"xt2_{grp}_{j}", name=f"xt2_{grp}_{j}")
                    nc.sync.dma_start(t[:], xt2_d[b])
                    xt2_t[grp][j] = t
                for j in range(GB):
                    b = grp * GB + j
                    t = ipool.tile([128, 16, IN_DIM], bf16, tag=f"x_{grp}_{j}", name=f"x_{grp}_{j}")
                    nc.sync.dma_start(t[:], x_d[b])
                    x_t[grp][j] = t

            nc.sync.dma_start(wsum_t[:], wsum_d[:])
            for grp in range(ng):
                xs_t[grp] = ipool.tile([IN_DIM, 128], bf16, tag=f"xs{grp}", name=f"xs{grp}")
                nc.sync.dma_start(xs_t[grp][:], xs_d[grp])
            load_group(0)
            # consts on the gpsimd queue so the sync queue stays on group data
            nc.gpsimd.dma_start(i128_t[:], i128_d[:])
            nc.gpsimd.dma_start(wt_t[:], wt_d[:])
            nc.gpsimd.dma_start(ef_t[:], ef_d[:])
            nc.gpsimd.dma_start(w2_t[:], w2_d[:])
            load_group(1)

            # pre-load the combined ln+exp+square act table (id 6,
            # natural_log_exp_and_others): the table-load pass then sees every
            # activation served and inserts no further ACT_TABLE_LOADs
            nc.scalar.add_instruction(mybir.InstLoadActFuncSet(
                name=nc.get_next_instruction_name(), ins=[], outs=[],
                act_func_set_id=6))

            eps_t = cpool.tile([128, 1], f32, tag="eps")
            nc.vector.memset(eps_t[:], EPS)
            # PE warmup: ~3.4us of dummy matmuls during the input-DMA wait so
            # the HAM clock gate reaches 8/8 (2.4 GHz) before real work starts
            warm_t = cpool.tile([IN_DIM, 128], bf16, tag="warm")
            nc.vector.memset(warm_t[:], 0.0)
            wz_ps = ps_z.tile([128, 4, 128], f32, tag="z")
            for wi in range(16):
                nc.tensor.matmul(
                    wz_ps[:, wi % 4, :], lhsT=warm_t[:], rhs=warm_t[:],
                    start=True, stop=True, skip_group_check=True,
                )
            one_eps_t = cpool.tile([128, 1], f32, tag="one_eps")
            nc.vector.memset(one_eps_t[:], 1.0 + EPS)

            # ---------------- squash ----------------
            def squash(o_ps, want_f32=False, for_transpose=True):
                """psum [128,64] -> (o_f32|None, o_raw bf16|None, diag(f)|None).

                f = sqrt(s')/(1+s') with u = exp(0.5*ln(s')) on Act while the
                1/(1+s') branch runs on DVE in parallel; all Act funcs live in
                the ln/exp/square table.  f is applied to the routing copy via
                diag(f) as the transpose identity (never on the o_raw copy,
                which starts right after mm2).
                """
                scr = wpool.tile([128, DIM], bf16, tag="scr")
                s0 = wpool.tile([128, 1], f32, tag="s0")
                nc.scalar.activation(scr[:], o_ps[:], Act.Square, accum_out=s0[:])
                o_raw = None
                if for_transpose:
                    o_raw = wpool.tile([128, DIM], bf16, tag="osqb")
                    nc.vector.tensor_copy(o_raw[:], o_ps[:])
                la = wpool.tile([128, 1], f32, tag="la")
                nc.scalar.activation(la[:], s0[:], Act.Ln, bias=eps_t[:])
                u = wpool.tile([128, 1], f32, tag="u")
                nc.scalar.activation(u[:], la[:], Act.Exp, scale=0.5)
                v = wpool.tile([128, 1], f32, tag="v")
                nc.vector.tensor_scalar_add(v[:], s0[:], 1.0 + EPS)
                rv = wpool.tile([128, 1], f32, tag="rv")
                nc.vector.reciprocal(rv[:], v[:])
                f = wpool.tile([128, 1], f32, tag="f")
                nc.vector.tensor_mul(f[:], u[:], rv[:])
                o_f32 = None
                if want_f32:
                    o_f32 = wpool.tile([128, DIM], f32, tag="osqf")
                    nc.scalar.activation(o_f32[:], o_ps[:], Act.Copy, scale=f[:])
                df = None
                if for_transpose:
                    df = wpool.tile([128, 128], bf16, tag="df")
                    nc.vector.tensor_scalar_mul(df[:], i128_t[:], f[:])
                return o_f32, o_raw, df

            def transpose_o(o_raw, df):
                """oT = (f*o)^T via a normal matmul: out[d, q] =
                sum_p o_raw[p, d] * diag(f)[p, q] = o[q, d] * f[q]."""
                t_ps = ps_wave.tile([IN_DIM, 128], f32, tag="pw")
                nc.tensor.matmul(t_ps[:], lhsT=o_raw[:], rhs=df[:],
                                 start=True, stop=True)
                oT = wpool.tile([IN_DIM, 128], bf16, tag="oT")
                nc.vector.tensor_copy(oT[:], t_ps[:])
                return oT

            def zstep(oT):
                """oT [64,128] -> z2 sbuf [64 i, 32 g, 128 p'] bf16 (single half;
                odd-parity db matmuls reach it via tile_position col packing)."""
                z2 = bigpool.tile([IN_DIM, 32, 128], bf16, tag="z2")
                for gw in range(8):  # waves of 4 g
                    z_ps = ps_z.tile([IN_DIM, 4, 128], f32, tag="z")
                    for j in range(4):
                        g = gw * 4 + j
                        nc.tensor.matmul(
                            z_ps[:, j, :], lhsT=wt_t[:, g, :], rhs=oT[:],
                            start=True, stop=True,
                        )
                    dst = z2[:, gw * 4 : gw * 4 + 4, :]
                    if gw % 2 == 0:
                        nc.scalar.copy(dst, z_ps[:])
                    else:
                        nc.vector.tensor_copy(dst, z_ps[:])
                return z2

            def dbstep(z2, xt2_g, b_prev):
                """-> new b sbuf [128, 32 g, 64 slot] f32 (written per wave)."""
                nb = bigpool.tile([128, 32, 64], f32, tag="b")
                for j in range(GB):  # one wave per batch: 16 slots
                    db_ps = ps_db.tile([128, 32, 16], f32, tag="db")
                    for kc in range(16):
                        slot = j * 16 + kc
                        for parity in range(2):
                            h = parity * 64
                            n = 2 * kc + parity
                            nc.tensor.matmul(
                                db_ps[h : h + 64, :, kc],
                                lhsT=xt2_g[j][:, n * 64 : (n + 1) * 64],
                                rhs=z2[:, :, h + slot],
                                start=True, stop=True,
                                tile_position=(0, h),
                            )
                    dst = nb[:, :, j * 16 : (j + 1) * 16]
                    if b_prev is None:
                        if j % 2 == 0:
                            nc.vector.tensor_copy(dst, db_ps[:])
                        else:
                            nc.scalar.copy(dst, db_ps[:])
                    else:
                        nc.vector.tensor_add(
                            dst, b_prev[:, :, j * 16 : (j + 1) * 16], db_ps[:]
                        )
                return nb

            def smp_step(b_sb, x_g):
                """softmax + pstep, wave-pipelined per batch j.

                b [128, 32 g, 64 slot] f32 -> p_all [128, 64 slot, 32 g] bf16.
                """
                expb = bigpool.tile([128, 32, 64], bf16, tag="expb")
                T = wpool.tile([128, GB, 32], bf16, tag="T")
                rs2 = wpool.tile([128, GB, 32], f32, tag="rs2")
                p_all = bigpool.tile([128, 32, 64], bf16, tag="pall")
                mul_eng = [nc.vector, nc.gpsimd, nc.vector, nc.gpsimd]
                for j in range(GB):
                    sl = slice(j * 16, (j + 1) * 16)
                    # exp of this batch's 16 slots (all g): [128, 32, 16]
                    nc.scalar.activation(expb[:, :, sl], b_sb[:, :, sl], Act.Exp)
                    # T[p, j, g] = sum_kc expb[p, g, j*16+kc]  (bf16 out: the
                    # 0.4% rounding on the softmax denominator is within budget
                    # and enables the 2x 16-bit DVE mode)
                    with nc.allow_low_precision(reason="softmax denom bf16"):
                        nc.vector.tensor_reduce(
                            T[:, j, :].unsqueeze(-1).squeeze(-1),
                            expb[:, :, sl],
                            mybir.AxisListType.X, Alu.add,
                        )
                    # S2[p, g] = sum over both parity halves, broadcast to
                    # all 128 partitions in one matmul
                    S2_ps = ps_sm.tile([128, 32], f32, tag="sm")
                    nc.tensor.matmul(
                        S2_ps[:], lhsT=ef_t[:], rhs=T[:, j, :],
                        start=True, stop=True,
                    )
                    nc.vector.reciprocal(rs2[:, j, :], S2_ps[:])
                    # c = expb * 1/S2 (broadcast over kc), in place
                    mul_eng[j].tensor_mul(
                        expb[:, :, sl],
                        expb[:, :, sl],
                        rs2[:, j, :, None].to_broadcast([128, 32, 16]),
                    )
                    # pstep for this batch's 16 slots (g-major psum so the
                    # mm2 weight slices come out contiguous)
                    p_ps = ps_wave.tile([128, 32, 16], f32, tag="pw")
                    for kc in range(16):
                        slot = j * 16 + kc
                        for parity in range(2):
                            h = parity * 64
                            nc.tensor.matmul(
                                p_ps[h : h + 64, :, kc],
                                lhsT=x_g[j][h : h + 64, kc, :],
                                rhs=expb[h : h + 64, :, slot],
                                start=True, stop=True,
                            )
                    if j % 2 == 0:
                        nc.scalar.copy(p_all[:, :, sl], p_ps[:])
                    else:
                        nc.vector.tensor_copy(p_all[:, :, sl], p_ps[:])
                return p_all

            def mm2(p_all, o_ps):
                # MM2: per g, row-packed even/odd halves accumulate into o-all.
                for g in range(32):
                    nc.tensor.matmul(
                        o_ps[0:64, :],
                        lhsT=p_all[0:64, g, :],
                        rhs=w2_t[0:64, g * 64 : (g + 1) * 64],
                        start=(g == 0), stop=(g == 31),
                        skip_group_check=True,
                    )
                    nc.tensor.matmul(
                        o_ps[64:128, :],
                        lhsT=p_all[64:128, g, :],
                        rhs=w2_t[64:128, g * 64 : (g + 1) * 64],
                        start=(g == 0), stop=(g == 31),
                        skip_group_check=True,
                    )

            # ================= interleaved group emission =================
            st = [dict() for _ in range(ng)]

            def ph_iter0(g_):
                s_ = st[g_]
                o_ps = ps_o.tile([128, DIM], f32, tag="o")
                nc.tensor.matmul(o_ps[:], lhsT=xs_t[g_][:], rhs=wsum_t[:], start=True, stop=True)
                _, s_["o_raw"], s_["df"] = squash(o_ps)
                s_["b"] = None
                s_["m2s_count"] = 0

            def ph_tz(g_):
                s_ = st[g_]
                oT = transpose_o(s_["o_raw"], s_["df"])
                s_["z2"] = zstep(oT)

            def ph_db(g_):
                s_ = st[g_]
                s_["b"] = dbstep(s_["z2"], xt2_t[g_], s_["b"])

            def ph_smp(g_):
                s_ = st[g_]
                s_["pall"] = smp_step(s_["b"], x_t[g_])

            def ph_m2s(g_):
                s_ = st[g_]
                o_ps = ps_o.tile([128, DIM], f32, tag="o")
                mm2(s_["pall"], o_ps)
                s_["m2s_count"] += 1
                final = s_["m2s_count"] == 2
                s_["o_f32"], s_["o_raw"], s_["df"] = squash(
                    o_ps, want_f32=final, for_transpose=not final)

            def ph_out(g_):
                grp, s_ = g_, st[g_]
                # one DMA per group: sbuf [128 (par, j, kc), 64] -> dram view
                dst = out_d[grp * GB : (grp + 1) * GB].rearrange(
                    "b (kc par) d -> par b kc d", par=2
                )
                nc.sync.dma_start(dst, s_["o_f32"][:])

            phases = [ph_iter0, ph_tz, ph_db, ph_smp, ph_m2s,
                      ph_tz, ph_db, ph_smp, ph_m2s, ph_out]
            OFFSET = 1
            for k in range(len(phases) + OFFSET * (ng - 1)):
                for grp in range(ng):
                    kk = k - OFFSET * grp
                    if 0 <= kk < len(phases):
                        phases[kk](grp)

    nc.compile()
    return nc


def _get_nc():
    if "nc" not in _CACHE:
        _CACHE["nc"] = _build_nc()
    return _CACHE["nc"]


def _prep_host_small(inputs, kern):
    """Host-side input prep; inputs [Bn, 2048, 64] with Bn a multiple of GB."""
    import ml_dtypes

    bf = ml_dtypes.bfloat16
    Bn = inputs.shape[0]
    ng = Bn // GB
    X = np.ascontiguousarray(inputs, dtype=np.float32)
    W = np.ascontiguousarray(kern.reshape(IN_DIM, NUM * DIM), dtype=np.float32)

    # x[b, parity*64+G, kc, i] = X[b, (2kc+parity)*64+G, i]
    xr = X.reshape(Bn, 16, 2, 64, IN_DIM)          # [b, kc, parity, G, i]
    x_h = np.ascontiguousarray(xr.transpose(0, 2, 3, 1, 4).reshape(Bn, 128, 16, IN_DIM))
    xt2_h = np.ascontiguousarray(X.transpose(0, 2, 1))  # [b, i=64, r]
    # xs[grp, i, parity*64 + j*16 + kc] = sum_G X[b, (2kc+parity)*64+G, i]
    xsum = X.reshape(Bn, 16, 2, 64, IN_DIM).sum(axis=3)  # [b, kc, parity, i]
    xs_h = np.zeros((ng, IN_DIM, 128), np.float32)
    for grp in range(ng):
        for j in range(GB):
            for parity in range(2):
                blk = xsum[grp * GB + j, :, parity, :].T
                xs_h[grp, :, parity * 64 + j * 16 : parity * 64 + (j + 1) * 16] = blk
    w2_h = np.concatenate([W, W], axis=0)          # [128, 2048]
    wt_h = np.ascontiguousarray(W.reshape(IN_DIM, 32, 64).transpose(2, 1, 0))  # [d, g, i]
    wsum_h = np.ascontiguousarray(W.reshape(IN_DIM, 32, 64).sum(axis=1) / 32.0)
    i128_h = np.eye(128, dtype=np.float32)
    eye64 = np.eye(64, dtype=np.float32)
    ef_h = np.tile(eye64, (2, 2))                  # [128, 128]
    return (
        x_h.astype(bf), xt2_h.astype(bf), xs_h.astype(bf),
        w2_h.astype(bf), wt_h.astype(bf), wsum_h.astype(bf), i128_h.astype(bf),
        ef_h.astype(bf),
    )


def _make_in_maps(inputs, kern):
    x_h, xt2_h, xs_h, w2_h, wt_h, wsum_h, i128_h, ef_h = _prep_host_small(
        np.asarray(inputs), np.asarray(kern)
    )
    in_maps = []
    for c in range(N_CORES):
        sl = slice(c * BPC, (c + 1) * BPC)
        gsl = slice(c * NG, (c + 1) * NG)
        in_maps.append(
            {
                "x": x_h[sl], "xt2": xt2_h[sl], "xs": xs_h[gsl],
                "w2": w2_h, "wt": wt_h, "wsum": wsum_h, "i128": i128_h,
                "ef": ef_h,
            }
        )
    return in_maps


def kernel(inputs, kernel, num_capsule=NUM, dim_capsule=DIM, routings=3, **_):
    from concourse.bass_utils import run_bass_kernel_spmd

    assert int(num_capsule) == NUM and int(dim_capsule) == DIM and int(routings) == 3
    nc = _get_nc()
    in_maps = _make_in_maps(inputs, kernel)
    res = run_bass_kernel_spmd(nc, in_maps, core_ids=list(range(N_CORES)))
    out = np.concatenate([res.results[c]["out"] for c in range(N_CORES)], axis=0)
    return out.astype(np.float32)
